# revision 6
# baseline (speedup 1.0000x reference)
"""Masked dot-product attention on 8 Trainium2 NeuronCores (Bass/Tile).

Problem: B=8, H=16, S=1024, D=64 attention where scores at key positions
k >= valid_lens[b] are masked to 1e-6 (not -inf) before softmax.

Sharding: core m gets batch m (16 heads + one valid_len) — no communication.

Host prep per core:
  - qT, kT: [H, D, S] pre-transposed (PE contracts over the partition dim, so
    both QK^T operands need D on partitions).
  - kT rows at k >= valid_len are zeroed: masked scores become exp(0)=1
    instead of exp(1e-6); relative difference 1e-6, far below fp32 tolerance.
    Masked keys' V rows still contribute (uniform weight), as the reference
    requires.

Device pipeline per head (everything fp32; matmuls cast to fp32r which runs
at full PE rate for free dim >= 256):
  1. scoresT[k, q] = K @ Q^T via lhsT=kT chunk [64,128], rhs=qT [64,512]
  2. pT = exp(0.125 * scoresT)  (ACT, scale folded into the activation)
  3. outT[d(+1), q] = sum_kc V_aug[kc].T @ pT[kc]  with a ones-column in
     V_aug producing the softmax denominator as row 64 for free
  4. PE-transpose outT back to [q, d+1]; reciprocal of col 64; per-partition
     scale; DMA out.
"""

import os
from contextlib import ExitStack

import numpy as np

import concourse.bass as bass
import concourse.mybir as mybir
import concourse.tile as tile
from concourse import bacc
from concourse.masks import make_identity

F32 = mybir.dt.float32
F32R = mybir.dt.float32r

B, H, S, D = 8, 16, 1024, 64
N_CORES = 8
KC = S // 128          # 8 key chunks of 128
QH = S // 512          # 2 query halves of 512
EXPF = mybir.ActivationFunctionType.Exp
SCALE = 1.0 / 8.0      # 1/sqrt(64)


def _emit_head(nc, pools, qT, kT, v, out, h, pending):
    """Emit instructions for head h. `pending` holds the deferred epilogue of
    the previous head; it is flushed mid-way through this head's QK stream so
    the PE never stalls waiting on the DVE evacuation."""
    (qk_pool, va_pool, pt_pool, pv_pool, ob_pool, sc_pool,
     ps_s_pool, ps_o_pool, ps_t_pool, identity) = pools

    qt = qk_pool.tile([64, S], F32R, tag="qt")
    nc.sync.dma_start(qt[:], qT[h])
    kt = qk_pool.tile([64, S], F32R, tag="kt")
    nc.sync.dma_start(kt[:], kT[h])
    va = va_pool.tile([128, KC, D + 1], F32R, tag="va")
    nc.sync.dma_start(va[:], v[h].rearrange("(kc p) d -> p kc d", p=128))

    ps_o = [ps_o_pool.tile([D + 1, 512], F32, tag="ps_o", name=f"ps_o{qh}")
            for qh in range(QH)]

    pt_prev = None
    for kc in range(KC):
        ps_s = ps_s_pool.tile([128, S], F32, tag="ps_s")
        for qh in range(QH):
            nc.tensor.matmul(
                ps_s[:, qh * 512:(qh + 1) * 512],
                lhsT=kt[:, kc * 128:(kc + 1) * 128],
                rhs=qt[:, qh * 512:(qh + 1) * 512],
                start=True, stop=True,
            )
        pt = pt_pool.tile([128, S], F32R, tag="pt")
        nc.scalar.activation(pt[:], ps_s[:], EXPF, scale=SCALE)

        if kc == 1 and pending:
            pending.pop()()
        if pt_prev is not None:
            _emit_pv(nc, ps_o, va, pt_prev, kc - 1)
        pt_prev = pt
    _emit_pv(nc, ps_o, va, pt_prev, KC - 1)

    # Evacuate PSUM on the DVE right away; defer the PE transposes etc.
    pvs = []
    for qh in range(QH):
        pv_sb = pv_pool.tile([D + 1, 512], F32, tag="pv")
        nc.vector.tensor_copy(pv_sb[:], ps_o[qh][:])
        pvs.append(pv_sb)

    def epilogue():
        for qh in range(QH):
            ps_t = ps_t_pool.tile([128, 4, D + 1], F32, tag="ps_t")
            for j in range(4):
                nc.tensor.transpose(
                    ps_t[:, j, :],
                    pvs[qh][:, j * 128:(j + 1) * 128],
                    identity[0:D + 1, 0:D + 1],
                )
            recip = sc_pool.tile([128, 4], F32, tag="recip")
            nc.vector.reciprocal(recip[:], ps_t[:, :, D])
            ob = ob_pool.tile([128, 4, D], F32, tag="ob")
            for j in range(4):
                nc.vector.tensor_scalar_mul(
                    ob[:, j, :], ps_t[:, j, 0:D], recip[:, j:j + 1])
            nc.sync.dma_start(
                out[h][qh * 512:(qh + 1) * 512, :].rearrange(
                    "(j p) d -> p j d", p=128),
                ob[:],
            )

    pending.append(epilogue)


def _emit_pv(nc, ps_o, va, pt, kc):
    for qh in range(QH):
        nc.tensor.matmul(
            ps_o[qh][:],
            lhsT=va[:, kc, :],
            rhs=pt[:, qh * 512:(qh + 1) * 512],
            start=(kc == 0), stop=(kc == KC - 1),
        )


def build_program(repeat: int = 1):
    nc = bacc.Bacc("TRN2", target_bir_lowering=False, debug=False,
                   enable_asserts=True, num_devices=N_CORES)
    qT = nc.dram_tensor("qT", [H, D, S], F32R, kind="ExternalInput").ap()
    kT = nc.dram_tensor("kT", [H, D, S], F32R, kind="ExternalInput").ap()
    v = nc.dram_tensor("v", [H, S, D + 1], F32R, kind="ExternalInput").ap()
    out = nc.dram_tensor("out", [H, S, D], F32, kind="ExternalOutput").ap()

    with tile.TileContext(nc) as tc:
        with ExitStack() as ctx:
            const_pool = ctx.enter_context(tc.tile_pool(name="const", bufs=1))
            identity = const_pool.tile([128, 128], F32)
            make_identity(nc, identity[:])

            pools = (
                ctx.enter_context(tc.tile_pool(name="qk", bufs=2)),
                ctx.enter_context(tc.tile_pool(name="va", bufs=2)),
                ctx.enter_context(tc.tile_pool(name="pt", bufs=3)),
                ctx.enter_context(tc.tile_pool(name="pv", bufs=4)),
                ctx.enter_context(tc.tile_pool(name="ob", bufs=3)),
                ctx.enter_context(tc.tile_pool(name="sc", bufs=4)),
                ctx.enter_context(tc.tile_pool(name="ps_s", bufs=2, space="PSUM")),
                ctx.enter_context(tc.tile_pool(name="ps_o", bufs=2, space="PSUM")),
                ctx.enter_context(tc.tile_pool(name="ps_t", bufs=2, space="PSUM")),
                identity,
            )
            pending = []
            for _ in range(repeat):
                for h in range(H):
                    _emit_head(nc, pools, qT, kT, v, out, h, pending)
            pending.pop()()
    nc.compile()
    return nc


_RUNNER_CACHE = {}


def _get_runner(repeat: int = 1):
    """Build the program once and return a cached jitted SPMD callable.

    Mirrors the multi-core tail of concourse.bass2jax.run_bass_via_pjrt, but
    caches the jitted function so repeat kernel() calls don't re-trace."""
    if repeat in _RUNNER_CACHE:
        return _RUNNER_CACHE[repeat]

    import jax
    from jax.sharding import Mesh, PartitionSpec
    from jax.experimental.shard_map import shard_map
    from concourse import bass2jax

    nc = build_program(repeat)
    bass2jax.install_neuronx_cc_hook()

    partition_name = (nc.partition_id_tensor.name
                      if nc.partition_id_tensor else None)
    in_names, out_names, out_avals, zero_outs = [], [], [], []
    for alloc in nc.m.functions[0].allocations:
        if not isinstance(alloc, mybir.MemoryLocationSet):
            continue
        name = alloc.memorylocations[0].name
        if alloc.kind == "ExternalInput":
            if name != partition_name:
                in_names.append(name)
        elif alloc.kind == "ExternalOutput":
            out_names.append(name)
            shape = tuple(alloc.tensor_shape)
            dtype = mybir.dt.np(alloc.dtype)
            out_avals.append(jax.core.ShapedArray(shape, dtype))
            zero_outs.append(np.zeros(shape, dtype))
    n_params = len(in_names)
    n_outs = len(out_avals)
    all_in_names = in_names + out_names
    if partition_name is not None:
        all_in_names = all_in_names + [partition_name]

    def _body(*args):
        operands = list(args)
        if partition_name is not None:
            operands.append(bass2jax.partition_id_tensor())
        outs = bass2jax._bass_exec_p.bind(
            *operands,
            out_avals=tuple(out_avals),
            in_names=tuple(all_in_names),
            out_names=tuple(out_names),
            lowering_input_output_aliases=(),
            sim_require_finite=True,
            sim_require_nnan=True,
            nc=nc,
        )
        return tuple(outs)

    devices = jax.devices()[:N_CORES]
    assert len(devices) == N_CORES, f"need {N_CORES} devices, got {len(devices)}"
    mesh = Mesh(np.asarray(devices), ("core",))
    donate = tuple(range(n_params, n_params + n_outs))
    sharded = jax.jit(
        shard_map(
            _body, mesh=mesh,
            in_specs=(PartitionSpec("core"),) * (n_params + n_outs),
            out_specs=(PartitionSpec("core"),) * n_outs,
            check_rep=False,
        ),
        donate_argnums=donate, keep_unused=True,
    )

    def run(in_maps):
        concat_in = [
            np.concatenate([m[name] for m in in_maps], axis=0)
            for name in in_names
        ]
        concat_zeros = [
            np.zeros((N_CORES * z.shape[0], *z.shape[1:]), z.dtype)
            for z in zero_outs
        ]
        out_arrs = sharded(*concat_in, *concat_zeros)
        return [
            {
                name: np.asarray(out_arrs[i]).reshape(
                    N_CORES, *out_avals[i].shape)[c]
                for i, name in enumerate(out_names)
            }
            for c in range(N_CORES)
        ]

    _RUNNER_CACHE[repeat] = (run, sharded, in_names, out_names, out_avals, nc)
    return _RUNNER_CACHE[repeat]


def make_in_maps(queries, keys, values, valid_lens):
    q = np.ascontiguousarray(np.asarray(queries, dtype=np.float32)).reshape(B, H, S, D)
    k = np.ascontiguousarray(np.asarray(keys, dtype=np.float32)).reshape(B, H, S, D)
    v = np.ascontiguousarray(np.asarray(values, dtype=np.float32)).reshape(B, H, S, D)
    vl = np.asarray(valid_lens).astype(np.int64).reshape(B)
    in_maps = []
    for m in range(B):
        km = k[m].copy()
        km[:, int(vl[m]):, :] = 0.0
        va = np.empty((H, S, D + 1), np.float32)
        va[:, :, :D] = v[m]
        va[:, :, D] = 1.0
        in_maps.append({
            "qT": np.ascontiguousarray(q[m].transpose(0, 2, 1)),
            "kT": np.ascontiguousarray(km.transpose(0, 2, 1)),
            "v": va,
        })
    return in_maps


def kernel(queries, keys, values, valid_lens):
    in_maps = make_in_maps(queries, keys, values, valid_lens)
    run = _get_runner(1)[0]
    results = run(in_maps)
    out = np.empty((B * H, S, D), dtype=np.float32)
    for m in range(N_CORES):
        out[m * H:(m + 1) * H] = results[m]["out"]
    return out


# revision 8
# speedup vs baseline: 7.7021x; 7.7021x over previous
"""Masked dot-product attention on 8 Trainium2 NeuronCores (Bass/Tile).

Problem: B=8, H=16, S=1024, D=64 attention where scores at key positions
k >= valid_lens[b] are masked to 1e-6 (not -inf) before softmax.

Sharding: core m gets batch m (16 heads + one valid_len) — no communication.

Host prep per core:
  - qT, kT: [H, D, S] pre-transposed (PE contracts over the partition dim, so
    both QK^T operands need D on partitions).
  - kT rows at k >= valid_len are zeroed: masked scores become exp(0)=1
    instead of exp(1e-6); relative difference 1e-6, far below fp32 tolerance.
    Masked keys' V rows still contribute (uniform weight), as the reference
    requires.

Device pipeline per head (everything fp32; matmuls cast to fp32r which runs
at full PE rate for free dim >= 256):
  1. scoresT[k, q] = K @ Q^T via lhsT=kT chunk [64,128], rhs=qT [64,512]
  2. pT = exp(0.125 * scoresT)  (ACT, scale folded into the activation)
  3. outT[d(+1), q] = sum_kc V_aug[kc].T @ pT[kc]  with a ones-column in
     V_aug producing the softmax denominator as row 64 for free
  4. PE-transpose outT back to [q, d+1]; reciprocal of col 64; per-partition
     scale; DMA out.
"""

import os
from contextlib import ExitStack

import numpy as np

import concourse.bass as bass
import concourse.mybir as mybir
import concourse.tile as tile
from concourse import bacc
from concourse.masks import make_identity

F32 = mybir.dt.float32
F32R = mybir.dt.float32r

B, H, S, D = 8, 16, 1024, 64
N_CORES = 8
KC = S // 128          # 8 key chunks of 128
QH = S // 512          # 2 query halves of 512
EXPF = mybir.ActivationFunctionType.Exp
SCALE = 1.0 / 8.0      # 1/sqrt(64)


def _emit_head(nc, pools, qT, kT, v, out, h, pending):
    """Emit instructions for head h. `pending` holds the deferred epilogue of
    the previous head; it is flushed mid-way through this head's QK stream so
    the PE never stalls waiting on the DVE evacuation."""
    (qk_pool, va_pool, pt_pool, pv_pool, ob_pool, sc_pool,
     ps_s_pool, ps_o_pool, ps_t_pool, identity) = pools

    qt = qk_pool.tile([64, S], F32R, tag="qt")
    nc.sync.dma_start(qt[:], qT[h])
    kt = qk_pool.tile([64, S], F32R, tag="kt")
    nc.sync.dma_start(kt[:], kT[h])
    va = va_pool.tile([128, KC, D + 1], F32R, tag="va")
    nc.sync.dma_start(va[:], v[h].rearrange("(kc p) d -> p kc d", p=128))

    ps_o = [ps_o_pool.tile([D + 1, 512], F32, tag="ps_o", name=f"ps_o{qh}")
            for qh in range(QH)]

    pt_prev = None
    for kc in range(KC):
        ps_s = ps_s_pool.tile([128, S], F32, tag="ps_s")
        for qh in range(QH):
            nc.tensor.matmul(
                ps_s[:, qh * 512:(qh + 1) * 512],
                lhsT=kt[:, kc * 128:(kc + 1) * 128],
                rhs=qt[:, qh * 512:(qh + 1) * 512],
                start=True, stop=True,
            )
        pt = pt_pool.tile([128, S], F32R, tag="pt")
        nc.scalar.activation(pt[:], ps_s[:], EXPF, scale=SCALE)

        if kc == 1 and pending:
            pending.pop()()
        if pt_prev is not None:
            _emit_pv(nc, ps_o, va, pt_prev, kc - 1)
        pt_prev = pt
    _emit_pv(nc, ps_o, va, pt_prev, KC - 1)

    # Evacuate PSUM on the DVE right away; defer the PE transposes etc.
    pvs = []
    for qh in range(QH):
        pv_sb = pv_pool.tile([D + 1, 512], F32, tag="pv")
        nc.vector.tensor_copy(pv_sb[:], ps_o[qh][:])
        pvs.append(pv_sb)

    def epilogue():
        for qh in range(QH):
            ps_t = ps_t_pool.tile([128, 4, D + 1], F32, tag="ps_t")
            for j in range(4):
                nc.tensor.transpose(
                    ps_t[:, j, :],
                    pvs[qh][:, j * 128:(j + 1) * 128],
                    identity[0:D + 1, 0:D + 1],
                )
            recip = sc_pool.tile([128, 4], F32, tag="recip")
            nc.vector.reciprocal(recip[:], ps_t[:, :, D])
            ob = ob_pool.tile([128, 4, D], F32, tag="ob")
            for j in range(4):
                nc.vector.tensor_scalar_mul(
                    ob[:, j, :], ps_t[:, j, 0:D], recip[:, j:j + 1])
            nc.sync.dma_start(
                out[h][qh * 512:(qh + 1) * 512, :].rearrange(
                    "(j p) d -> p j d", p=128),
                ob[:],
            )

    pending.append(epilogue)


def _emit_pv(nc, ps_o, va, pt, kc):
    for qh in range(QH):
        nc.tensor.matmul(
            ps_o[qh][:],
            lhsT=va[:, kc, :],
            rhs=pt[:, qh * 512:(qh + 1) * 512],
            start=(kc == 0), stop=(kc == KC - 1),
        )


def build_program(repeat: int = 1, loop: int = 1):
    nc = bacc.Bacc("TRN2", target_bir_lowering=False, debug=False,
                   enable_asserts=True, num_devices=N_CORES)
    qT = nc.dram_tensor("qT", [H, D, S], F32R, kind="ExternalInput").ap()
    kT = nc.dram_tensor("kT", [H, D, S], F32R, kind="ExternalInput").ap()
    v = nc.dram_tensor("v", [H, S, D + 1], F32R, kind="ExternalInput").ap()
    out = nc.dram_tensor("out", [H, S, D], F32, kind="ExternalOutput").ap()

    with tile.TileContext(nc) as tc:
        with ExitStack() as ctx:
            const_pool = ctx.enter_context(tc.tile_pool(name="const", bufs=1))
            identity = const_pool.tile([128, 128], F32)
            make_identity(nc, identity[:])

            pools = (
                ctx.enter_context(tc.tile_pool(name="qk", bufs=2)),
                ctx.enter_context(tc.tile_pool(name="va", bufs=2)),
                ctx.enter_context(tc.tile_pool(name="pt", bufs=3)),
                ctx.enter_context(tc.tile_pool(name="pv", bufs=4)),
                ctx.enter_context(tc.tile_pool(name="ob", bufs=3)),
                ctx.enter_context(tc.tile_pool(name="sc", bufs=4)),
                ctx.enter_context(tc.tile_pool(name="ps_s", bufs=2, space="PSUM")),
                ctx.enter_context(tc.tile_pool(name="ps_o", bufs=2, space="PSUM")),
                ctx.enter_context(tc.tile_pool(name="ps_t", bufs=2, space="PSUM")),
                identity,
            )

            def body(_i=None):
                pending = []
                for _ in range(repeat):
                    for h in range(H):
                        _emit_head(nc, pools, qT, kT, v, out, h, pending)
                pending.pop()()

            if loop == 1:
                body()
            else:
                with tc.For_i(0, loop, 1):
                    body()
    nc.compile()
    return nc


_RUNNER_CACHE = {}


def _get_runner(repeat: int = 1, loop: int = 1):
    """Build the program once and return a cached jitted SPMD callable.

    Mirrors the multi-core tail of concourse.bass2jax.run_bass_via_pjrt, but
    caches the jitted function so repeat kernel() calls don't re-trace."""
    key = (repeat, loop)
    if key in _RUNNER_CACHE:
        return _RUNNER_CACHE[key]

    import jax
    from jax.sharding import Mesh, PartitionSpec
    from jax.experimental.shard_map import shard_map
    from concourse import bass2jax

    nc = build_program(repeat, loop)
    bass2jax.install_neuronx_cc_hook()

    partition_name = (nc.partition_id_tensor.name
                      if nc.partition_id_tensor else None)
    in_names, out_names, out_avals, zero_outs = [], [], [], []
    for alloc in nc.m.functions[0].allocations:
        if not isinstance(alloc, mybir.MemoryLocationSet):
            continue
        name = alloc.memorylocations[0].name
        if alloc.kind == "ExternalInput":
            if name != partition_name:
                in_names.append(name)
        elif alloc.kind == "ExternalOutput":
            out_names.append(name)
            shape = tuple(alloc.tensor_shape)
            dtype = mybir.dt.np(alloc.dtype)
            out_avals.append(jax.core.ShapedArray(shape, dtype))
            zero_outs.append(np.zeros(shape, dtype))
    n_params = len(in_names)
    n_outs = len(out_avals)
    all_in_names = in_names + out_names
    if partition_name is not None:
        all_in_names = all_in_names + [partition_name]

    def _body(*args):
        operands = list(args)
        if partition_name is not None:
            operands.append(bass2jax.partition_id_tensor())
        outs = bass2jax._bass_exec_p.bind(
            *operands,
            out_avals=tuple(out_avals),
            in_names=tuple(all_in_names),
            out_names=tuple(out_names),
            lowering_input_output_aliases=(),
            sim_require_finite=True,
            sim_require_nnan=True,
            nc=nc,
        )
        return tuple(outs)

    devices = jax.devices()[:N_CORES]
    assert len(devices) == N_CORES, f"need {N_CORES} devices, got {len(devices)}"
    mesh = Mesh(np.asarray(devices), ("core",))
    donate = tuple(range(n_params, n_params + n_outs))
    sharded = jax.jit(
        shard_map(
            _body, mesh=mesh,
            in_specs=(PartitionSpec("core"),) * (n_params + n_outs),
            out_specs=(PartitionSpec("core"),) * n_outs,
            check_rep=False,
        ),
        donate_argnums=donate, keep_unused=True,
    )

    def run(in_maps):
        concat_in = [
            np.concatenate([m[name] for m in in_maps], axis=0)
            for name in in_names
        ]
        concat_zeros = [
            np.zeros((N_CORES * z.shape[0], *z.shape[1:]), z.dtype)
            for z in zero_outs
        ]
        out_arrs = sharded(*concat_in, *concat_zeros)
        return [
            {
                name: np.asarray(out_arrs[i]).reshape(
                    N_CORES, *out_avals[i].shape)[c]
                for i, name in enumerate(out_names)
            }
            for c in range(N_CORES)
        ]

    def make_dev_args(in_maps):
        """Ship inputs to the devices once; returns (dev_in, fresh_zeros_fn)."""
        from jax.sharding import NamedSharding
        sh = NamedSharding(mesh, PartitionSpec("core"))
        concat_in = [
            np.concatenate([m[name] for m in in_maps], axis=0)
            for name in in_names
        ]
        dev_in = [jax.device_put(a, sh) for a in concat_in]
        jax.block_until_ready(dev_in)

        def fresh_zeros():
            zs = [jax.device_put(
                np.zeros((N_CORES * z.shape[0], *z.shape[1:]), z.dtype), sh)
                for z in zero_outs]
            jax.block_until_ready(zs)
            return zs

        return dev_in, fresh_zeros

    _RUNNER_CACHE[key] = (run, sharded, make_dev_args, out_names, out_avals, nc)
    return _RUNNER_CACHE[key]


def make_in_maps(queries, keys, values, valid_lens):
    q = np.ascontiguousarray(np.asarray(queries, dtype=np.float32)).reshape(B, H, S, D)
    k = np.ascontiguousarray(np.asarray(keys, dtype=np.float32)).reshape(B, H, S, D)
    v = np.ascontiguousarray(np.asarray(values, dtype=np.float32)).reshape(B, H, S, D)
    vl = np.asarray(valid_lens).astype(np.int64).reshape(B)
    in_maps = []
    for m in range(B):
        km = k[m].copy()
        km[:, int(vl[m]):, :] = 0.0
        va = np.empty((H, S, D + 1), np.float32)
        va[:, :, :D] = v[m]
        va[:, :, D] = 1.0
        in_maps.append({
            "qT": np.ascontiguousarray(q[m].transpose(0, 2, 1)),
            "kT": np.ascontiguousarray(km.transpose(0, 2, 1)),
            "v": va,
        })
    return in_maps


def kernel(queries, keys, values, valid_lens):
    in_maps = make_in_maps(queries, keys, values, valid_lens)
    run = _get_runner(1)[0]
    results = run(in_maps)
    out = np.empty((B * H, S, D), dtype=np.float32)
    for m in range(N_CORES):
        out[m * H:(m + 1) * H] = results[m]["out"]
    return out


# revision 9
# speedup vs baseline: 10.8570x; 1.4096x over previous
"""Masked dot-product attention on 8 Trainium2 NeuronCores (Bass/Tile).

Problem: B=8, H=16, S=1024, D=64 attention where scores at key positions
k >= valid_lens[b] are masked to 1e-6 (not -inf) before softmax:
masked keys still contribute V with a uniform (unnormalized) weight of
exp(1e-6) ~= 1.

Sharding (SPMD, one program on 8 cores): each core takes 2 heads from EVERY
batch (core m gets heads b*16 + 2m, b*16 + 2m + 1). Since the masked length
is per-batch, every core sees the identical per-slot workload vector
[C_0, C_0, C_1, C_1, ..., C_7, C_7] where C_b = min(8, L_b//128 + 1) is the
number of 128-row key chunks that must be computed densely. The program is
specialized to that vector (compile cached per distinct valid_lens).

Masking, exactly:
  - kT rows with k >= L are zeroed on the host: their scores become exactly 0
    and their unnormalized weight exp(0) = 1 (vs exp(1e-6) in the reference:
    rel diff 1e-6, far below fp32 tolerance).
  - chunks >= C_b are skipped entirely; every skipped row would have weight
    exactly 1, so the host folds sum_{k >= C_b*128} [V[k], 1] into the
    (always masked) last row of the boundary chunk's V_aug. This is exact.

Device pipeline per head slot (fp32; matmuls in fp32r = full PE rate at
free dim >= 256, ~1.6e-4 max rel err measured on HW):
  1. scoresT[k, q] = K @ Q^T    (lhsT = kT chunk [64,128], rhs = qT [64,512])
  2. pT = exp(0.125 * scoresT)  (ACT, PSUM->SBUF, scale folded in)
  3. outT[d(+1), q] += V_aug[kc].T @ pT[kc]   (ones-column of V_aug makes
     row 64 the softmax denominator for free)
  4. PE-transpose outT to [q, d+1]; DVE reciprocal + per-partition scale; DMA.
"""

from contextlib import ExitStack

import numpy as np

import concourse.bass as bass  # noqa: F401
import concourse.mybir as mybir
import concourse.tile as tile
from concourse import bacc
from concourse.masks import make_identity

F32 = mybir.dt.float32
F32R = mybir.dt.float32r

B, H, S, D = 8, 16, 1024, 64
N_CORES = 8
HPC = H // N_CORES     # heads per (core, batch) = 2
KC = S // 128          # key chunks per full head
QH = S // 512          # query halves
EXPF = mybir.ActivationFunctionType.Exp
SCALE = 1.0 / 8.0      # 1/sqrt(64)

DENSE_CVEC = (KC,) * B


def _emit_head(nc, pools, qT, kT, v, out, h, C, pending):
    """Emit one head slot with C dense key chunks. `pending` holds deferred
    epilogues (PE transposes) of previous heads, flushed after this head's
    early QK work so the PE never stalls on the DVE evacuation."""
    (qk_pool, va_pool, pt_pool, pv_pool, ob_pool, sc_pool,
     ps_s_pool, ps_o_pool, ps_t_pool, identity) = pools

    qt = qk_pool.tile([64, S], F32R, tag="qt")
    nc.sync.dma_start(qt[:], qT[h])
    kt = qk_pool.tile([64, C * 128], F32R, tag="kt")
    nc.sync.dma_start(kt[:], kT[h][:, 0:C * 128])
    va = va_pool.tile([128, C, D + 1], F32R, tag="va")
    nc.sync.dma_start(
        va[:], v[h][0:C * 128].rearrange("(kc p) d -> p kc d", p=128))

    ps_o = [ps_o_pool.tile([D + 1, 512], F32, tag="ps_o", name=f"ps_o{qh}")
            for qh in range(QH)]

    flush_at = min(1, C - 1)
    pt_prev = None
    for kc in range(C):
        ps_s = ps_s_pool.tile([128, S], F32, tag="ps_s")
        for qh in range(QH):
            nc.tensor.matmul(
                ps_s[:, qh * 512:(qh + 1) * 512],
                lhsT=kt[:, kc * 128:(kc + 1) * 128],
                rhs=qt[:, qh * 512:(qh + 1) * 512],
                start=True, stop=True,
            )
        pt = pt_pool.tile([128, S], F32R, tag="pt")
        nc.scalar.activation(pt[:], ps_s[:], EXPF, scale=SCALE)

        if kc == flush_at:
            while pending:
                pending.pop(0)()
        if pt_prev is not None:
            _emit_pv(nc, ps_o, va, pt_prev, kc - 1, C)
        pt_prev = pt
    _emit_pv(nc, ps_o, va, pt_prev, C - 1, C)

    # Evacuate PSUM on the DVE right away; defer the PE work.
    pvs = []
    for qh in range(QH):
        pv_sb = pv_pool.tile([D + 1, 512], F32, tag="pv")
        nc.vector.tensor_copy(pv_sb[:], ps_o[qh][:])
        pvs.append(pv_sb)

    def epilogue():
        for qh in range(QH):
            ps_t = ps_t_pool.tile([128, 4, D + 1], F32, tag="ps_t")
            for j in range(4):
                nc.tensor.transpose(
                    ps_t[:, j, :],
                    pvs[qh][:, j * 128:(j + 1) * 128],
                    identity[0:D + 1, 0:D + 1],
                )
            recip = sc_pool.tile([128, 4], F32, tag="recip")
            nc.vector.reciprocal(recip[:], ps_t[:, :, D])
            ob = ob_pool.tile([128, 4, D], F32, tag="ob")
            for j in range(4):
                nc.vector.tensor_scalar_mul(
                    ob[:, j, :], ps_t[:, j, 0:D], recip[:, j:j + 1])
            nc.sync.dma_start(
                out[h][qh * 512:(qh + 1) * 512, :].rearrange(
                    "(j p) d -> p j d", p=128),
                ob[:],
            )

    pending.append(epilogue)


def _emit_pv(nc, ps_o, va, pt, kc, C):
    for qh in range(QH):
        nc.tensor.matmul(
            ps_o[qh][:],
            lhsT=va[:, kc, :],
            rhs=pt[:, qh * 512:(qh + 1) * 512],
            start=(kc == 0), stop=(kc == C - 1),
        )


def build_program(cvec=DENSE_CVEC, loop: int = 1, repeat: int = 1):
    """One SPMD program; head slot s (0..15) covers batch s//2 with
    cvec[s//2] dense chunks."""
    nc = bacc.Bacc("TRN2", target_bir_lowering=False, debug=False,
                   enable_asserts=True, num_devices=N_CORES)
    qT = nc.dram_tensor("qT", [H, D, S], F32R, kind="ExternalInput").ap()
    kT = nc.dram_tensor("kT", [H, D, S], F32R, kind="ExternalInput").ap()
    v = nc.dram_tensor("v", [H, S, D + 1], F32R, kind="ExternalInput").ap()
    out = nc.dram_tensor("out", [H, S, D], F32, kind="ExternalOutput").ap()

    with tile.TileContext(nc) as tc:
        with ExitStack() as ctx:
            const_pool = ctx.enter_context(tc.tile_pool(name="const", bufs=1))
            identity = const_pool.tile([128, 128], F32)
            make_identity(nc, identity[:])

            pools = (
                ctx.enter_context(tc.tile_pool(name="qk", bufs=2)),
                ctx.enter_context(tc.tile_pool(name="va", bufs=2)),
                ctx.enter_context(tc.tile_pool(name="pt", bufs=3)),
                ctx.enter_context(tc.tile_pool(name="pv", bufs=4)),
                ctx.enter_context(tc.tile_pool(name="ob", bufs=3)),
                ctx.enter_context(tc.tile_pool(name="sc", bufs=4)),
                ctx.enter_context(tc.tile_pool(name="ps_s", bufs=2, space="PSUM")),
                ctx.enter_context(tc.tile_pool(name="ps_o", bufs=2, space="PSUM")),
                ctx.enter_context(tc.tile_pool(name="ps_t", bufs=2, space="PSUM")),
                identity,
            )

            def body(_i=None):
                pending = []
                for _ in range(repeat):
                    for h in range(H):
                        _emit_head(nc, pools, qT, kT, v, out, h,
                                   cvec[h // HPC], pending)
                while pending:
                    pending.pop(0)()

            if loop == 1:
                body()
            else:
                with tc.For_i(0, loop, 1):
                    body()
    nc.compile()
    return nc


def cvec_of(valid_lens):
    vl = np.asarray(valid_lens).astype(np.int64).reshape(B)
    return tuple(int(min(KC, L // 128 + 1)) for L in vl)


def make_in_maps(queries, keys, values, valid_lens):
    """Per-core inputs: core m's head slot 2b+j holds head (b, 2m+j)."""
    q = np.ascontiguousarray(
        np.asarray(queries, dtype=np.float32)).reshape(B, H, S, D)
    k = np.ascontiguousarray(
        np.asarray(keys, dtype=np.float32)).reshape(B, H, S, D)
    v = np.ascontiguousarray(
        np.asarray(values, dtype=np.float32)).reshape(B, H, S, D)
    vl = np.asarray(valid_lens).astype(np.int64).reshape(B)
    cvec = cvec_of(vl)

    # [B, H, D+1, ...] staging with mask + fold applied per batch.
    km = k.copy()
    va = np.empty((B, H, S, D + 1), np.float32)
    va[..., :D] = v
    va[..., D] = 1.0
    for b in range(B):
        L, C = int(vl[b]), cvec[b]
        km[b, :, L:, :] = 0.0
        if C < KC:
            # Skipped rows all have unnormalized weight exactly 1; fold their
            # V_aug sum into the (masked) last row of the boundary chunk.
            va[b, :, C * 128 - 1, :] += va[b, :, C * 128:, :].sum(axis=1)

    qT = q.transpose(0, 1, 3, 2)   # [B, H, D, S]
    kT = km.transpose(0, 1, 3, 2)

    in_maps = []
    for m in range(N_CORES):
        hsel = [2 * m + j for j in range(HPC)]
        in_maps.append({
            "qT": np.ascontiguousarray(
                qT[:, hsel].reshape(H, D, S)),
            "kT": np.ascontiguousarray(
                kT[:, hsel].reshape(H, D, S)),
            "v": np.ascontiguousarray(
                va[:, hsel].reshape(H, S, D + 1)),
        })
    return in_maps, cvec


def scatter_outputs(results):
    """Inverse of the head assignment: full [B*H, S, D] from per-core outs."""
    out = np.empty((B, H, S, D), dtype=np.float32)
    for m in range(N_CORES):
        o = results[m].reshape(B, HPC, S, D)
        for j in range(HPC):
            out[:, 2 * m + j] = o[:, j]
    return out.reshape(B * H, S, D)


_NC_CACHE = {}


def _get_nc(cvec, loop=1, repeat=1):
    key = (cvec, loop, repeat)
    if key not in _NC_CACHE:
        _NC_CACHE[key] = build_program(cvec, loop, repeat)
    return _NC_CACHE[key]


def kernel(queries, keys, values, valid_lens):
    from concourse.bass_utils import run_bass_kernel_spmd

    in_maps, cvec = make_in_maps(queries, keys, values, valid_lens)
    nc = _get_nc(cvec)
    res = run_bass_kernel_spmd(nc, in_maps, list(range(N_CORES)))
    return scatter_outputs([res.results[m]["out"] for m in range(N_CORES)])


# ----------------------------------------------------------------------------
# Cached jitted runner (used by test.py for timing; avoids per-call re-trace
# and ships inputs to the devices once).
# ----------------------------------------------------------------------------
_RUNNER_CACHE = {}


def _get_runner(cvec=DENSE_CVEC, loop: int = 1):
    key = (cvec, loop)
    if key in _RUNNER_CACHE:
        return _RUNNER_CACHE[key]

    import jax
    from jax.sharding import Mesh, PartitionSpec, NamedSharding
    from jax.experimental.shard_map import shard_map
    from concourse import bass2jax

    nc = _get_nc(cvec, loop)
    bass2jax.install_neuronx_cc_hook()

    partition_name = (nc.partition_id_tensor.name
                      if nc.partition_id_tensor else None)
    in_names, out_names, out_avals, zero_outs = [], [], [], []
    for alloc in nc.m.functions[0].allocations:
        if not isinstance(alloc, mybir.MemoryLocationSet):
            continue
        name = alloc.memorylocations[0].name
        if alloc.kind == "ExternalInput":
            if name != partition_name:
                in_names.append(name)
        elif alloc.kind == "ExternalOutput":
            out_names.append(name)
            shape = tuple(alloc.tensor_shape)
            dtype = mybir.dt.np(alloc.dtype)
            out_avals.append(jax.core.ShapedArray(shape, dtype))
            zero_outs.append(np.zeros(shape, dtype))
    n_params = len(in_names)
    n_outs = len(out_avals)
    all_in_names = in_names + out_names
    if partition_name is not None:
        all_in_names = all_in_names + [partition_name]

    def _body(*args):
        operands = list(args)
        if partition_name is not None:
            operands.append(bass2jax.partition_id_tensor())
        outs = bass2jax._bass_exec_p.bind(
            *operands,
            out_avals=tuple(out_avals),
            in_names=tuple(all_in_names),
            out_names=tuple(out_names),
            lowering_input_output_aliases=(),
            sim_require_finite=True,
            sim_require_nnan=True,
            nc=nc,
        )
        return tuple(outs)

    devices = jax.devices()[:N_CORES]
    mesh = Mesh(np.asarray(devices), ("core",))
    donate = tuple(range(n_params, n_params + n_outs))
    sharded = jax.jit(
        shard_map(
            _body, mesh=mesh,
            in_specs=(PartitionSpec("core"),) * (n_params + n_outs),
            out_specs=(PartitionSpec("core"),) * n_outs,
            check_rep=False,
        ),
        donate_argnums=donate, keep_unused=True,
    )

    def run(in_maps):
        concat_in = [
            np.concatenate([m[name] for m in in_maps], axis=0)
            for name in in_names
        ]
        concat_zeros = [
            np.zeros((N_CORES * z.shape[0], *z.shape[1:]), z.dtype)
            for z in zero_outs
        ]
        out_arrs = sharded(*concat_in, *concat_zeros)
        return [
            {
                name: np.asarray(out_arrs[i]).reshape(
                    N_CORES, *out_avals[i].shape)[c]
                for i, name in enumerate(out_names)
            }
            for c in range(N_CORES)
        ]

    def make_dev_args(in_maps):
        sh = NamedSharding(mesh, PartitionSpec("core"))
        concat_in = [
            np.concatenate([m[name] for m in in_maps], axis=0)
            for name in in_names
        ]
        dev_in = [jax.device_put(a, sh) for a in concat_in]
        jax.block_until_ready(dev_in)

        def fresh_zeros():
            zs = [jax.device_put(
                np.zeros((N_CORES * z.shape[0], *z.shape[1:]), z.dtype), sh)
                for z in zero_outs]
            jax.block_until_ready(zs)
            return zs

        return dev_in, fresh_zeros

    _RUNNER_CACHE[key] = (run, sharded, make_dev_args, out_names, out_avals, nc)
    return _RUNNER_CACHE[key]
